# revision 52
# baseline (speedup 1.0000x reference)
"""Trainium2 8-core kernel for LN + RoPE multi-head attention + out-proj.

Sharding: tensor-parallel over heads. Core c owns heads (2c, 2c+1) = inner dims
[128c, 128c+128). Each core computes LN(x) @ its w_qkv column-slice, RoPE,
full-sequence attention for its 2 heads, and a partial out-projection against
its w_out row-slice. Host sums the 8 partial outputs and adds b_out.

v4 structure (477us baseline -> this):
- Host prep folds the LN affine exactly (all in fp32, more accurate than the
  on-device bf16 version): gamma into w_qkv; mean-centering into x itself;
  rstd into premultiplied cosR/sinR RoPE tables ([128, 8192] bf16, 2MB each)
  for q/k and a per-partition [128, 64] f32 layout for v. x is shipped
  pre-transposed (d-major) so QKV matmuls read it directly — no on-chip
  transposes of x, no LN stats pass, no (x-mu)*rstd elementwise pass.
- Phase A per 512-token tile is just: DMA xt -> 3x8 QKV matmuls -> RoPE
  (perm matmul + 3 TTs vs the rstd-premultiplied tables) -> v transpose with
  a per-partition rstd tensor_scalar.
- Phase B keeps the flat software-pipelined stream: one global sequence of
  256 (unit, jc) attention steps, scores for step g+2 emitted before the
  probs@v of step g, both heads' scores share one 2-bank psum via
  tile_position row-split, softmax denominators ride the pv matmul as a 65th
  ones-column of v, reciprocal on a [128, 8] DMA repack, and phase-A slices
  drip-fed between steps (emission order defines dependencies).
- Bulk DMA (xt, cosR/sinR) rides the gpsimd queue; the sync queue carries
  only small latency-sensitive transfers and the output tiles.
"""

import os
import sys

sys.path.insert(0, "/opt/trn_rl_repo")

import numpy as np

B, N, D = 4, 2048, 1024
H, HD = 16, 64
TOK = B * N  # 8192
NCORES = 8
EPS = 1e-5

_CACHE = {}


def _np_bf16():
    import ml_dtypes

    return ml_dtypes.bfloat16


def build_bass():
    import concourse.bass as bass
    import concourse.mybir as mybir
    import concourse.tile as tile
    from concourse import bacc
    from concourse.masks import make_identity

    f32 = mybir.dt.float32
    bf16 = mybir.dt.bfloat16
    AF = mybir.ActivationFunctionType
    ALU = mybir.AluOpType

    nc = bacc.Bacc("TRN2", target_bir_lowering=False, debug=False, num_devices=NCORES)

    # xt pre-tiled on host: [tile, half, 128, 4, 512] so each half-tile DMA
    # is one fully contiguous 512KB read (8KB per partition).
    xt_d = nc.dram_tensor("xt", [16 * 2 * 128, 4 * 512], bf16, kind="ExternalInput").ap()
    wqkv_d = nc.dram_tensor("wqkv", [128, 8 * 384], bf16, kind="ExternalInput").ap()
    perm_d = nc.dram_tensor("perm", [128, 128], bf16, kind="ExternalInput").ap()
    wout_d = nc.dram_tensor("wout", [128, D], bf16, kind="ExternalInput").ap()
    # RoPE tables pre-tiled: [tile*128, 512] so each tile chunk is contiguous
    cosr_d = nc.dram_tensor("cosR", [16 * 128, 512], bf16, kind="ExternalInput").ap()
    sinr_d = nc.dram_tensor("sinR", [16 * 128, 512], bf16, kind="ExternalInput").ap()
    rstdpp_d = nc.dram_tensor("rstdpp", [128, 64], f32, kind="ExternalInput").ap()
    e2_d = nc.dram_tensor("e2", [2, 128], bf16, kind="ExternalInput").ap()
    out_d = nc.dram_tensor("out", [TOK, D], bf16, kind="ExternalOutput").ap()

    with tile.TileContext(nc) as tc:
        with (
            tc.tile_pool(name="singles", bufs=1) as singles,
            tc.tile_pool(name="xtp", bufs=3) as xt_pool,
            tc.tile_pool(name="rtmp", bufs=4) as rtmp,
            tc.tile_pool(name="vst", bufs=4) as vst_pool,
            tc.tile_pool(name="probs", bufs=4) as pr_pool,
            tc.tile_pool(name="tmph", bufs=4) as tmph_pool,
            tc.tile_pool(name="norm", bufs=2) as norm_pool,
            tc.tile_pool(name="ost", bufs=3) as ost_pool,
            tc.tile_pool(name="ob", bufs=3) as ob_pool,
            tc.tile_pool(name="psA", bufs=2, space="PSUM") as ps_a,
            tc.tile_pool(name="psS", bufs=2, space="PSUM") as ps_st,
            tc.tile_pool(name="psOS", bufs=2, space="PSUM") as ps_os_pool,
        ):
            # ---- tile-0 inputs first: they gate the whole pipeline ----
            cosR_sb = singles.tile([128, TOK], bf16)
            sinR_sb = singles.tile([128, TOK], bf16)

            def dma_x_tile(tt, xt8):
                def xthalf(hf):
                    r0 = (tt * 2 + hf) * 128
                    nc.gpsimd.dma_start(
                        out=xt8[:, hf * 4 : (hf + 1) * 4, :],
                        in_=xt_d[r0 : r0 + 128, :].rearrange(
                            "p (c t) -> p c t", c=4
                        ),
                    )

                xthalf(0)
                nc.gpsimd.dma_start(
                    out=cosR_sb[:, tt * 512 : (tt + 1) * 512],
                    in_=cosr_d[tt * 128 : (tt + 1) * 128, :],
                )
                nc.gpsimd.dma_start(
                    out=sinR_sb[:, tt * 512 : (tt + 1) * 512],
                    in_=sinr_d[tt * 128 : (tt + 1) * 128, :],
                )
                xthalf(1)

            xt8_0 = xt_pool.tile([128, 8, 512], bf16, name="xt8")
            dma_x_tile(0, xt8_0)

            # ---- constants / persistent tiles (wqkv first, q slice first) ----
            wqkv_sb = singles.tile([128, 3, 8, 128], bf16)
            for f in range(3):
                nc.sync.dma_start(
                    out=wqkv_sb[:, f],
                    in_=wqkv_d[:, f * 1024 : (f + 1) * 1024].rearrange(
                        "p (c w) -> p c w", c=8
                    ),
                )
            ident = singles.tile([128, 128], bf16)
            make_identity(nc, ident)
            perm_sb = singles.tile([128, 128], bf16)
            nc.sync.dma_start(out=perm_sb, in_=perm_d)
            rstd_pp = singles.tile([128, 64], f32)
            nc.sync.dma_start(out=rstd_pp, in_=rstdpp_d)
            e2 = singles.tile([2, 128], bf16)
            nc.sync.dma_start(out=e2, in_=e2_d)
            wout_sb = singles.tile([128, D], bf16)
            nc.sync.dma_start(out=wout_sb, in_=wout_d)

            qT = singles.tile([128, TOK], bf16)  # rows: head0 dims 0-63, head1 64-127
            kT = singles.tile([128, TOK], bf16)
            v_sb = singles.tile([128, 64, 2, 65], bf16)  # [j, jchunk, head, 64v+1]
            nc.vector.memset(v_sb[:, :, :, 64:65], 1.0)

            # ---- phase A: QKV -> RoPE (rstd premultiplied into tables) ----
            def phase_a_gen(tt):  # 512-token tiles
                t0 = tt * 512
                if tt == 0:
                    xt8 = xt8_0
                else:
                    xt8 = xt_pool.tile([128, 8, 512], bf16, name="xt8")
                    dma_x_tile(tt, xt8)
                    yield
                # QKV projections: f= 0:q 1:k 2:v  (x pre-centered on host)
                for f in (0, 1, 2):
                    ps_q = ps_a.tile([128, 512], f32, tag="psA")
                    for dc in range(8):
                        nc.tensor.matmul(
                            ps_q,
                            wqkv_sb[:, f, dc, :],
                            xt8[:, dc, :],
                            start=dc == 0,
                            stop=dc == 7,
                        )
                        if dc == 3:
                            yield
                    if f == 2:
                        # v: transpose to token-major into v_sb, folding rstd
                        vstage = vst_pool.tile([128, 512], bf16)
                        nc.vector.tensor_copy(out=vstage, in_=ps_q)
                        ptv = ps_a.tile([128, 512], bf16, tag="psA")
                        for st in range(4):
                            nc.tensor.transpose(
                                ptv[:, st * 128 : (st + 1) * 128],
                                vstage[:, st * 128 : (st + 1) * 128],
                                ident,
                            )
                        for st in range(4):
                            nc.vector.tensor_scalar(
                                out=v_sb[:, tt * 4 + st, :, 0:64],
                                in0=ptv[:, st * 128 : (st + 1) * 128].rearrange(
                                    "p (h d) -> p h d", h=2
                                ),
                                scalar1=rstd_pp[:, tt * 4 + st : tt * 4 + st + 1],
                                scalar2=None,
                                op0=ALU.mult,
                            )
                    else:
                        # rope: dst = q*cosR + (perm.T @ q)*sinR  (cosR/sinR
                        # carry rstd, so dst is the fully normalized head)
                        q_sb = rtmp.tile([128, 512], bf16, tag="qsb")
                        nc.vector.tensor_copy(out=q_sb, in_=ps_q)
                        ps_qsw = ps_a.tile([128, 512], f32, tag="psA")
                        nc.tensor.matmul(
                            ps_qsw, perm_sb, q_sb, start=True, stop=True
                        )
                        a = rtmp.tile([128, 512], bf16, tag="ra")
                        nc.vector.tensor_tensor(
                            out=a, in0=q_sb, in1=cosR_sb[:, t0 : t0 + 512], op=ALU.mult
                        )
                        bt = rtmp.tile([128, 512], bf16, tag="rb")
                        nc.vector.tensor_tensor(
                            out=bt, in0=ps_qsw, in1=sinR_sb[:, t0 : t0 + 512], op=ALU.mult
                        )
                        dst = qT if f == 0 else kT
                        nc.vector.tensor_tensor(
                            out=dst[:, t0 : t0 + 512],
                            in0=a,
                            in1=bt,
                            op=ALU.add,
                        )
                        yield

            # ---- phase B: scores -> softmax -> probs@v -> normalize -> out ----
            unit_state = {}

            def emit_scores(b, it, jc):
                i0 = b * 2048 + it * 512
                j0 = b * 2048 + jc * 128
                ps_s = ps_st.tile([128, 1024], f32, tag="psS")
                for h in range(2):
                    hb = h * 64
                    nc.tensor.matmul(
                        ps_s[:, h * 512 : (h + 1) * 512],
                        kT[hb : hb + 64, j0 : j0 + 128],
                        qT[hb : hb + 64, i0 : i0 + 512],
                        start=True,
                        stop=True,
                        tile_position=(hb, 0),
                    )
                probs = pr_pool.tile([128, 1024], bf16)
                nc.scalar.activation(probs, ps_s, AF.Exp, scale=HD ** -0.5)
                unit_state.setdefault((b, it), {"probs": {}})["probs"][jc] = probs

            def emit_v(b, it, jc):
                st = unit_state[(b, it)]
                if jc == 0:
                    st["ps_os"] = [
                        ps_os_pool.tile(
                            [65, 512], f32, tag="psOS", name=f"ps_o_{b}_{it}_{h}"
                        )
                        for h in range(2)
                    ]
                jcg = b * 16 + jc
                probs = st["probs"].pop(jc)
                for h in range(2):
                    nc.tensor.matmul(
                        st["ps_os"][h],
                        v_sb[:, jcg, h, :],
                        probs[:, h * 512 : (h + 1) * 512],
                        start=jc == 0,
                        stop=jc == 15,
                    )

            pending_out = []

            def emit_tail(b, it, last=False):
                """Normalize by sum-exp and out-project one 512-query tile.
                The 1MB output DMA is deferred (pending_out) so the NEXT
                unit's small repack DMAs never queue behind it."""
                i0 = b * 2048 + it * 512
                qdma = nc.gpsimd.dma_start if last else nc.sync.dma_start
                ps_os = unit_state.pop((b, it))["ps_os"]
                tmpA = tmph_pool.tile([65, 512], bf16, tag="tmpA")
                nc.vector.tensor_copy(out=tmpA, in_=ps_os[0])
                tmpB = tmph_pool.tile([65, 512], bf16, tag="tmpB")
                if last:
                    nc.scalar.activation(out=tmpB, in_=ps_os[1], func=AF.Copy)
                else:
                    nc.vector.tensor_copy(out=tmpB, in_=ps_os[1])
                # denominators: repack the two 512-wide rows into a [128, 8]
                # tile via DMA so DVE reciprocal runs on 8 elems/partition.
                rpk = norm_pool.tile([128, 8], bf16, tag="rpk")
                nc.sync.dma_start(out=rpk[:, 0:4], in_=tmpA[64:65, :])
                nc.gpsimd.dma_start(out=rpk[:, 4:8], in_=tmpB[64:65, :])
                rpk2 = norm_pool.tile([128, 8], f32, tag="rpk2")
                nc.vector.reciprocal(out=rpk2, in_=rpk)
                rpk3 = norm_pool.tile([128, 8], bf16, tag="rpk3")
                nc.vector.tensor_copy(out=rpk3, in_=rpk2)
                rbf2 = norm_pool.tile([2, 512], bf16, tag="rbf2")
                nc.sync.dma_start(out=rbf2[0:1, :], in_=rpk3[:, 0:4])
                nc.gpsimd.dma_start(out=rbf2[1:2, :], in_=rpk3[:, 4:8])
                ostack = ost_pool.tile([128, 512], bf16)
                qdma(out=ostack[64:128, :], in_=tmpB[0:64, :])
                # For the last unit, the bc broadcast / normalize / out-proj
                # runs in two pipelined 256-token halves so the second half's
                # broadcast overlaps the first half's out-projection.
                halves = (0, 1) if last else (0,)
                hw_ = 256 if last else 512
                ob_big = ob_pool.tile([128, 4, D], bf16)
                for hv in halves:
                    c0 = hv * hw_
                    ps_bc = ps_a.tile([128, hw_], f32, tag="psA", name=f"psbc{hw_}")
                    nc.tensor.matmul(
                        ps_bc, e2, rbf2[:, c0 : c0 + hw_], start=True, stop=True
                    )
                    bc = norm_pool.tile([128, 512], bf16, tag="bc")
                    nc.scalar.activation(
                        out=bc[:, c0 : c0 + hw_], in_=ps_bc, func=AF.Copy
                    )
                    nc.vector.tensor_tensor(
                        out=ostack[0:64, c0 : c0 + hw_],
                        in0=tmpA[0:64, c0 : c0 + hw_],
                        in1=bc[0:64, c0 : c0 + hw_],
                        op=ALU.mult,
                    )
                    nc.vector.tensor_tensor(
                        out=ostack[64:128, c0 : c0 + hw_],
                        in0=ostack[64:128, c0 : c0 + hw_],
                        in1=bc[64:128, c0 : c0 + hw_],
                        op=ALU.mult,
                    )
                    for t4 in range(hv * 2, hv * 2 + 2) if last else range(4):
                        for Dc in range(2):
                            ps_op = ps_a.tile([128, 512], f32, tag="psA")
                            nc.tensor.matmul(
                                ps_op,
                                ostack[:, t4 * 128 : (t4 + 1) * 128],
                                wout_sb[:, Dc * 512 : (Dc + 1) * 512],
                                start=True,
                                stop=True,
                            )
                            if Dc == 1 and (last or t4 % 2 == 0):
                                nc.scalar.activation(
                                    out=ob_big[:, t4, Dc * 512 : (Dc + 1) * 512],
                                    in_=ps_op,
                                    func=AF.Copy,
                                )
                            else:
                                nc.vector.tensor_copy(
                                    out=ob_big[:, t4, Dc * 512 : (Dc + 1) * 512],
                                    in_=ps_op,
                                )
                        if last:
                            dq = nc.sync.dma_start if t4 % 2 else nc.gpsimd.dma_start
                            dq(
                                out=out_d[
                                    i0 + t4 * 128 : i0 + (t4 + 1) * 128, :
                                ].rearrange("(t p) d -> p t d", p=128),
                                in_=ob_big[:, t4 : t4 + 1, :],
                            )
                if not last:
                    def _emit_out(i0=i0, ob_big=ob_big):
                        nc.sync.dma_start(
                            out=out_d[i0 : i0 + 512, :].rearrange(
                                "(t p) d -> p t d", p=128
                            ),
                            in_=ob_big,
                        )
                    pending_out.append(_emit_out)

            # Flat software-pipelined stream over 256 (unit, jc) steps.
            units = [(b, it) for b in range(4) for it in range(4)]
            PIPE = 2

            def run_a(tt):
                for _ in phase_a_gen(tt):
                    pass

            run_a(0)
            pro_gens = [phase_a_gen(1), phase_a_gen(2), phase_a_gen(3)]

            def drive_pro():
                while pro_gens:
                    try:
                        next(pro_gens[0])
                        return
                    except StopIteration:
                        pro_gens.pop(0)

            def drain_pro(n_left):
                while len(pro_gens) > n_left:
                    try:
                        next(pro_gens[0])
                    except StopIteration:
                        pro_gens.pop(0)

            b0, it0 = units[0]
            for jc in range(16):
                if jc == 4:
                    drain_pro(2)  # tile 1 fully emitted
                elif jc == 8:
                    drain_pro(1)  # tile 2
                elif jc == 12:
                    drain_pro(0)  # tile 3
                emit_scores(b0, it0, jc)
                if jc >= PIPE:
                    emit_v(b0, it0, jc - PIPE)
                drive_pro()
                drive_pro()

            a_queue = [phase_a_gen(4 + k) for k in range(4)]

            def drive_a():
                while a_queue:
                    try:
                        next(a_queue[0])
                        return
                    except StopIteration:
                        a_queue.pop(0)

            NSTEP = 16 * len(units)
            for g in range(16, NSTEP + PIPE):
                if g % 64 == 0 and g < NSTEP:
                    b = g // 64
                    if b < 3:
                        a_queue.extend(phase_a_gen(4 * (b + 1) + k) for k in range(4))
                if g < NSTEP:
                    u, jc = divmod(g, 16)
                    emit_scores(*units[u], jc)
                if g >= PIPE:
                    u, jc = divmod(g - PIPE, 16)
                    emit_v(*units[u], jc)
                    if jc == 15:
                        emit_tail(*units[u], last=u == len(units) - 1)
                gm = g % 16
                if gm == 7 and pending_out:
                    pending_out.pop(0)()
                if gm in (15, 0, 1):
                    pass  # unit-boundary steps are the tightest-gated
                elif g % 2 == 0 or gm in (5, 9):
                    drive_a()

    nc.finalize()
    return nc


def make_in_maps(x, ln_gamma, ln_beta, w_qkv):
    bf = _np_bf16()
    x = np.asarray(x, np.float32).reshape(TOK, D)
    mu = x.mean(axis=1, keepdims=True)
    var = x.var(axis=1)
    rstd = 1.0 / np.sqrt(var + EPS)  # [TOK] fp32
    xc = (x - mu).astype(bf)
    # [tile, half, 128p, 4chunk, 512tok] with d = (half*4 + chunk)*128 + p
    xt = np.ascontiguousarray(
        xc.T.reshape(2, 4, 128, 16, 512)  # [half, chunk, p, tile, tok]
        .transpose(3, 0, 2, 1, 4)  # [tile, half, p, chunk, tok]
        .reshape(16 * 2 * 128, 4 * 512)
    )
    g = np.asarray(ln_gamma, np.float32)
    w = np.asarray(w_qkv, np.float32)
    w_eff = g[:, None] * w  # [D, 3*INNER]

    # rope tables with rstd premultiplied (position within batch = tok % N)
    inv_freq = 1.0 / (10000.0 ** (np.arange(0, HD, 2, dtype=np.float32) / HD))
    pos = np.arange(N, dtype=np.float32)
    ang = pos[:, None] * inv_freq[None, :]  # [N, 32]
    cosT = np.cos(ang).T.astype(np.float32)  # [32, N]
    sinT = np.sin(ang).T.astype(np.float32)
    cos128 = np.tile(cosT, (4, 1))  # [128, N], rows p -> cos[p%32]
    sin128s = np.tile(sinT, (4, 1))
    sin128s[0:32] *= -1.0
    sin128s[64:96] *= -1.0
    postab = np.tile(cos128, (1, B))  # [128, TOK]
    sintab = np.tile(sin128s, (1, B))
    cosR = (postab * rstd[None, :]).astype(bf)
    sinR = (sintab * rstd[None, :]).astype(bf)
    # pre-tile: [tile*128, 512] so each tile chunk is one contiguous read
    cosR = np.ascontiguousarray(
        cosR.reshape(128, 16, 512).transpose(1, 0, 2).reshape(16 * 128, 512)
    )
    sinR = np.ascontiguousarray(
        sinR.reshape(128, 16, 512).transpose(1, 0, 2).reshape(16 * 128, 512)
    )
    # per-partition rstd for v: rstdpp[p, tt*4+st] = rstd[tt*512 + st*128 + p]
    rstdpp = np.ascontiguousarray(
        rstd.reshape(16, 4, 128).transpose(2, 0, 1).reshape(128, 64)
    ).astype(np.float32)

    perm_np = np.zeros((128, 128), np.float32)
    for p in range(128):
        sig = (p % 64 + 32) % 64 + 64 * (p // 64)
        perm_np[sig, p] = 1.0
    perm_np = perm_np.astype(bf)

    e2_np = np.zeros((2, 128), np.float32)
    e2_np[0, 0:64] = 1.0
    e2_np[1, 64:128] = 1.0
    e2_np = e2_np.astype(bf)

    in_maps = []
    for c in range(NCORES):
        sl = slice(128 * c, 128 * c + 128)
        wq = w_eff[:, 0:1024][:, sl]
        wk = w_eff[:, 1024:2048][:, sl]
        wv = w_eff[:, 2048:3072][:, sl]

        wcat = np.concatenate([wq, wk, wv], axis=1).astype(bf)  # [D, 384]
        # device layout [p, f, c, 128]: col = f*1024 + c*128 + w
        wqkv_t = np.ascontiguousarray(
            wcat.reshape(8, 128, 3, 128).transpose(1, 2, 0, 3).reshape(128, 8 * 384)
        )
        in_maps.append(
            {
                "xt": xt,
                "wqkv": wqkv_t,
                "cosR": cosR,
                "sinR": sinR,
                "rstdpp": rstdpp,
                "wout": None,  # filled below by caller (needs w_out)
                "e2": e2_np,
                "perm": perm_np,
            }
        )
    return in_maps


def _run(inputs, trace=False):
    from concourse import bass_utils

    if "nc" not in _CACHE:
        _CACHE["nc"] = build_bass()
    nc = _CACHE["nc"]

    bf = _np_bf16()
    w_out = np.asarray(inputs["w_out"], np.float32)
    b_out = np.asarray(inputs["b_out"], np.float32)
    beta = np.asarray(inputs["ln_beta"], np.float32)
    assert np.allclose(beta, 0.0, atol=1e-12), "nonzero ln_beta unsupported"

    in_maps = make_in_maps(
        inputs["x"], inputs["ln_gamma"], inputs["ln_beta"], inputs["w_qkv"]
    )
    for c in range(NCORES):
        in_maps[c]["wout"] = np.ascontiguousarray(
            w_out[128 * c : 128 * c + 128, :].astype(bf)
        )

    res = bass_utils.run_bass_kernel_spmd(
        nc, in_maps, core_ids=list(range(NCORES)), trace=trace
    )
    total = np.zeros((TOK, D), np.float32)
    for r in res.results:
        total += np.asarray(r["out"], np.float32)
    total += b_out[None, :]
    return total.reshape(B, N, D), res


def kernel(**inputs):
    out, _ = _run(inputs, trace=False)
    return out
